# revision 26
# baseline (speedup 1.0000x reference)
"""CP-adapter multi-head attention on 8 Trainium2 NeuronCores.

Hardcoded for B=4, N=2048, D=1024, H=16, hd=64, R=r=64 (fp32 in/out).

Sharding: tensor-parallel over heads.  Core c owns heads (2c, 2c+1) =
columns [128c, 128c+128) of the q/k/v projections and rows [128c, 128c+128)
of the output projection; every core streams the full activations (bf16).
Each core emits a partial output [8192, 1024] bf16; the host sums the 8
partials in f32 and adds the bias (the only cross-core reduction).

Kernel design:
- The CP adapter is linear (dropout p=0), so it folds ON HOST into
  effective weights: W_eff = W + U @ cp @ V, cp = einsum(CP_C, CP_attn).
  The device sees only pre-layouted bf16 weight tiles (<0.5% of FLOPs).
- All matmuls run in bf16 (1 cycle/row of the moving operand, f32
  accumulate in PSUM).  Streams and intermediates are bf16, which halves
  HBM traffic vs f32.
- q/k are produced transposed ([cols, tokens], W_eff stationary / X^T
  moving); v is produced transposed then PE-transposed to natural layout
  with ones columns appended for the softmax denominator.
- Attention per (batch, q-quarter): scores computed transposed,
  ST = K^T-block stationary x Q^T moving, two heads row-tiled into one
  PSUM stripe [128 keys, 2x512 q] per key block; one Exp per stripe on
  ScalarE (free scale=1/8); PV accumulates [V|1] stationary over key
  blocks into O' [65, 512] whose row 64 is the softmax denominator.
- Software pipeline with lookahead: ST(kb+1) is issued BEFORE PV(kb) so
  the in-order PE keeps ScalarE fed back-to-back (PV waits on exp(kb);
  without the lookahead it would block ST(kb+1) and starve ScalarE).
- Fill work (next-batch qkv quarters, deferred projection tiles) is
  sliced into ~430ns sub-units and woven between attention units so the
  PE never idles and the HAM clock stays at full speed.
"""

from contextlib import ExitStack

import numpy as np
import ml_dtypes

try:
    import concourse.bass as bass
except ImportError:  # fallback when sitecustomize paths are absent
    import sys
    sys.path.append("/opt/trn_rl_repo")
    import concourse.bass as bass
import concourse.mybir as mybir
from concourse import bacc, tile
from concourse.bass_utils import run_bass_kernel_spmd
from concourse.masks import make_identity

F32 = mybir.dt.float32
BF16 = mybir.dt.bfloat16
AF = mybir.ActivationFunctionType

B, N, D = 4, 2048, 1024
H, HD = 16, 64
R = 64
NCORES = 8
TOK = B * N            # 8192
CPB = D // NCORES      # 128 cols per core
ATT_SCALE = HD ** -0.5

BF16_NP = ml_dtypes.bfloat16
DEBUG_DUMPS = False


def _build():
    nc = bacc.Bacc(None, target_bir_lowering=False, debug=False)

    # ---- external inputs (per-core views prepared on host) ----
    xqT = nc.dram_tensor("xqT", [16, 128, 4096], BF16, kind="ExternalInput")
    xkT = nc.dram_tensor("xkT", [16, 128, 4096], BF16, kind="ExternalInput")
    xvT = nc.dram_tensor("xvT", [16, 128, 4096], BF16, kind="ExternalInput")
    # effective weights, CP adapter folded on host, pre-layouted:
    # w_eff[t] [ki, ko, j]: W_eff[(ko*128+ki), col_c + j]
    weffq_d = nc.dram_tensor("weffq", [128, 8, CPB], BF16, kind="ExternalInput")
    weffk_d = nc.dram_tensor("weffk", [128, 8, CPB], BF16, kind="ExternalInput")
    weffv_d = nc.dram_tensor("weffv", [128, 8, CPB], BF16, kind="ExternalInput")
    # proj rows for this core: [ki, ch, j]: Wp_eff[row_c + ki, ch*512 + j]
    weffp_d = nc.dram_tensor("weffp", [CPB, 2, 512], BF16, kind="ExternalInput")

    out = nc.dram_tensor("out", [TOK, D], BF16, kind="ExternalOutput")
    if DEBUG_DUMPS:
        dbg_q = nc.dram_tensor("dbg_q", [128, 4, 512], BF16, kind="ExternalOutput")
        dbg_k = nc.dram_tensor("dbg_k", [128, 4, 512], BF16, kind="ExternalOutput")
        dbg_vb = nc.dram_tensor("dbg_vb", [128, 16, 130], BF16,
                                kind="ExternalOutput")
        dbg_pt = nc.dram_tensor("dbg_pt", [128, 1024], BF16, kind="ExternalOutput")
        dbg_osb = nc.dram_tensor("dbg_osb", [65, 512], F32, kind="ExternalOutput")
        dbg_xa = nc.dram_tensor("dbg_xa", [128, 4, 512], BF16,
                                kind="ExternalOutput")

    xT3 = {"q": xqT, "k": xkT, "v": xvT}
    weff_dram = {"q": weffq_d, "k": weffk_d, "v": weffv_d}

    with tile.TileContext(nc) as tc:
        with ExitStack() as es:
            const = es.enter_context(tc.tile_pool(name="const", bufs=1))
            weffp_pool = es.enter_context(tc.tile_pool(name="weff", bufs=1))
            xstream = es.enter_context(tc.tile_pool(name="xstream", bufs=16))
            qkvp = es.enter_context(tc.tile_pool(name="qkv", bufs=2))
            ptp = es.enter_context(tc.tile_pool(name="pt", bufs=3))
            normp = es.enter_context(tc.tile_pool(name="norm", bufs=4))
            outst = es.enter_context(tc.tile_pool(name="outst", bufs=3))
            ps_qkv = es.enter_context(
                tc.tile_pool(name="ps_qkv", bufs=2, space="PSUM"))
            ps_st = es.enter_context(
                tc.tile_pool(name="ps_st", bufs=2, space="PSUM"))
            ps_o = es.enter_context(
                tc.tile_pool(name="ps_o", bufs=2, space="PSUM"))

            # ---------- weights + constants ----------
            # weff DMAs are issued lazily (interleaved with the first xs
            # streams in the startup section) so the k-projection chain
            # reaches the first exp as early as possible.
            weff = {}

            def load_weff(t):
                we = weffp_pool.tile([128, 8, CPB], BF16, name=f"weff{t}")
                nc.sync.dma_start(we[:], weff_dram[t][:])
                weff[t] = we
            weff_p = weffp_pool.tile([CPB, 2, 512], BF16)
            identf = const.tile([128, 128], F32)
            make_identity(nc, identf)
            ident = const.tile([128, 128], BF16)
            nc.vector.tensor_copy(ident[:], identf[:])
            onesf = const.tile([128, 1], F32)
            nc.any.memset(onesf[:], 1.0)

            # ---------- qkv fill sub-units ----------
            def qkv_units(b):
                qkvT = {
                    "q": qkvp.tile([128, 4, 512], BF16, name="qT", tag="qT"),
                    "k": qkvp.tile([128, 4, 512], BF16, name="kT", tag="kT"),
                }
                v_b = qkvp.tile([128, 16, 130], BF16, name="v_b", tag="v_b")
                state = (qkvT, v_b)

                def ones_unit():
                    nc.vector.tensor_copy(
                        v_b[:, :, 64:65],
                        onesf[:, None, :].broadcast_to([128, 16, 1]))
                    nc.vector.tensor_copy(
                        v_b[:, :, 129:130],
                        onesf[:, None, :].broadcast_to([128, 16, 1]))

                def mm_sub(t, tb, dp, holder):
                    # quarter of a projection tile: one xs DMA + 2 matmuls
                    # (~430ns of PE) so fill work can slot into every
                    # exp-wait gap and the PE clock never ramps down.
                    def f():
                        if dp == 0:
                            holder["ps"] = ps_qkv.tile(
                                [128, 512], F32, name="psqkv", tag="psqkv")
                        ps = holder["ps"]
                        xs = xstream.tile([128, 1024], BF16,
                                          name="xs", tag="xs")
                        nc.sync.dma_start(
                            xs[:],
                            xT3[t][b * 4 + tb, :,
                                   dp * 1024:(dp + 1) * 1024])
                        for j in range(2):
                            db = dp * 2 + j
                            nc.tensor.matmul(
                                ps[:], weff[t][:, db, :],
                                xs[:, j * 512:(j + 1) * 512],
                                start=(db == 0), stop=(db == 7))
                        if dp < 3:
                            return
                        if t == "v":
                            vt = outst.tile([128, 512], BF16, name="vt",
                                            tag="vt")
                            nc.vector.tensor_copy(vt[:], ps[:])
                            for j in range(4):
                                kb = tb * 4 + j
                                tp = ps_qkv.tile([128, 128], BF16,
                                                 name="pstr", tag="psqkv")
                                nc.tensor.transpose(
                                    tp[:], vt[:, j * 128:(j + 1) * 128],
                                    ident[:])
                                nc.vector.tensor_copy(v_b[:, kb, 0:64],
                                                      tp[:, 0:64])
                                nc.vector.tensor_copy(v_b[:, kb, 65:129],
                                                      tp[:, 64:128])
                        else:
                            nc.vector.tensor_copy(qkvT[t][:, tb, :], ps[:])
                    return f

                units = [ones_unit]
                for t in ("k", "q", "v"):
                    for tb in range(4):
                        holder = {}
                        for dp in range(4):
                            units.append(mm_sub(t, tb, dp, holder))
                return state, units

            # ---------- attention ----------
            def attn_units(b, state):
                qkvT, v_b = state
                xaT = qkvp.tile([128, 4, 512], BF16, name="xaT", tag="xaT")
                sts, pvs, norms = [], [], []

                def group(qq):
                    o_ps = [
                        ps_o.tile([65, 512], F32, name="o_ps", tag="o_ps")
                        for _ in range(2)
                    ]
                    pt_holder = {}

                    def st_unit(kb):
                        def f():
                            st = ps_st.tile([128, 1024], F32, name="st",
                                            tag="st")
                            ktb, ksub = kb // 4, kb % 4
                            for hh in range(2):
                                ro = hh * 64
                                nc.tensor.matmul(
                                    st[:, hh * 512:(hh + 1) * 512],
                                    qkvT["k"][ro:ro + 64, ktb,
                                              ksub * 128:(ksub + 1) * 128],
                                    qkvT["q"][ro:ro + 64, qq, :],
                                    start=True, stop=True)
                            pt = ptp.tile([128, 1024], BF16, name="pt",
                                          tag="pt")
                            nc.scalar.activation(pt[:], st[:], AF.Exp,
                                                 scale=ATT_SCALE)
                            pt_holder[kb] = pt
                        return f

                    def pv_unit(kb):
                        def f():
                            pt = pt_holder[kb]
                            for hh in range(2):
                                nc.tensor.matmul(
                                    o_ps[hh][:],
                                    v_b[:, kb, hh * 65:(hh + 1) * 65],
                                    pt[:, hh * 512:(hh + 1) * 512],
                                    start=(kb == 0), stop=(kb == 15))
                        return f

                    def norm_unit():
                        # Stage O' (and both denominators) out of PSUM right
                        # away so o_ps frees fast and the next group's PV can
                        # start; one reciprocal covers both heads
                        # (denominators at partitions 0 and 32 -- partition
                        # starts must be 32-aligned).
                        o_sb = [None, None]
                        for hh in range(2):
                            o_sb[hh] = normp.tile([64, 512], F32,
                                                  name=f"o_sb{hh}",
                                                  tag=f"o_sb{hh}")
                            nc.vector.tensor_copy(o_sb[hh][:],
                                                  o_ps[hh][0:64, :])
                        d2 = normp.tile([64, 512], F32, name="d2", tag="d2")
                        nc.gpsimd.memset(d2[:], 1.0)
                        nc.vector.tensor_copy(d2[0:1, :], o_ps[0][64:65, :])
                        nc.vector.tensor_copy(d2[32:33, :], o_ps[1][64:65, :])
                        rec2 = normp.tile([64, 512], F32, name="rec2",
                                          tag="rec2")
                        nc.vector.reciprocal(rec2[:], d2[:])
                        rec_b = normp.tile([1, 512], F32, name="rec_b",
                                           tag="rec_b")
                        nc.vector.tensor_copy(rec_b[:], rec2[32:33, :])
                        for hh in range(2):
                            ro = hh * 64
                            rec64 = normp.tile([64, 512], F32, name="rec64",
                                               tag="rec64")
                            nc.gpsimd.partition_broadcast(
                                rec64[:],
                                rec2[0:1, :] if hh == 0 else rec_b[:])
                            nc.vector.tensor_mul(xaT[ro:ro + 64, qq, :],
                                                 o_sb[hh][:], rec64[:])

                    sts.extend(st_unit(kb) for kb in range(16))
                    pvs.extend(pv_unit(kb) for kb in range(16))
                    norms.append(norm_unit)

                for qq in range(4):
                    group(qq)
                return xaT, sts, pvs, norms

            def proj_units(b, xaT, qq):
                tok0 = b * N

                def tb_unit(tb):
                    def f():
                        sub = tb % 4
                        lx = xaT[:, qq, sub * 128:(sub + 1) * 128]
                        ob = outst.tile([128, 1024], BF16, name="ob", tag="ob")
                        for ch in range(2):
                            ps = ps_qkv.tile([128, 512], F32, name="pspj",
                                             tag="psqkv")
                            nc.tensor.matmul(ps[:], lx, weff_p[:, ch, :],
                                             start=True, stop=True)
                            nc.vector.tensor_copy(
                                ob[:, ch * 512:(ch + 1) * 512], ps[:])
                        nc.sync.dma_start(
                            out[tok0 + tb * 128:tok0 + (tb + 1) * 128, :],
                            ob[:])
                    return f
                return [tb_unit(qq * 4 + j) for j in range(4)]

            def batch_stream(sts, pvs, norms):
                # Global lookahead order: S0 S1 [P0 S2] [P1 S3] ... with
                # norm(qq) right after P(16qq+15).  ST(i+2) immediately
                # follows PV(i), so the in-order PE never lets ScalarE
                # starve -- including across group boundaries.
                su = [("S", sts[0]), ("S", sts[1])]
                for i in range(64):
                    if i + 2 < 64:
                        su.append(("S", sts[i + 2]))
                    su.append(("P", pvs[i]))
                    if i % 16 == 15:
                        su.append(("N", norms[i // 16]))
                return su

            # ---------- batch 0 startup ----------
            state, units0 = qkv_units(0)
            # units0 layout: [ones, k(16 subs), q(16 subs), v(16 subs)]
            ones_u = units0[0]
            k_u = lambda tb: units0[1 + 4 * tb:5 + 4 * tb]
            q_u = lambda tb: units0[17 + 4 * tb:21 + 4 * tb]
            v_u = lambda tb: units0[33 + 4 * tb:37 + 4 * tb]
            ones_u()
            load_weff("k")
            for u in k_u(0):
                u()
            load_weff("q")
            for u in q_u(0):
                u()
            load_weff("v")
            for u in v_u(0):
                u()
            nc.sync.dma_start(weff_p[:], weffp_d[:])

            # ---------- main pipeline ----------
            # proj units of group g are deferred into the unit stream of the
            # NEXT group, so the PE never waits on the normalization chain
            # at a group boundary.
            pending = []
            for b in range(B):
                xaT, sts, pvs, norms = attn_units(b, state)
                fill = []
                if b + 1 < B:
                    state, fill = qkv_units(b + 1)
                fi = 0
                stream = batch_stream(sts, pvs, norms)
                sub_q = []
                p_cnt = 0
                n_cnt = 0
                for si, (kind, u) in enumerate(stream):
                    if b == 0 and si in (0, 6, 14):
                        tbn = {0: 1, 6: 2, 14: 3}[si]
                        sub_q += k_u(tbn) + v_u(tbn) + q_u(tbn)
                    u()
                    if sub_q:
                        sub_q.pop(0)()
                    if kind == "P":
                        pi = p_cnt % 16
                        p_cnt += 1
                        if pi in (5, 7, 9, 11) and pending:
                            pending.pop(0)()
                        elif pi >= 14:
                            pass  # keep DVE clear for the o_ps release
                        elif sub_q:
                            sub_q.pop(0)()
                        elif fi < len(fill):
                            fill[fi]()
                            fi += 1
                    elif kind == "N":
                        pending += proj_units(b, xaT, n_cnt)
                        n_cnt += 1
                        if fi < len(fill):
                            fill[fi]()
                            fi += 1
                for u in sub_q:
                    u()
                for u in fill[fi:]:
                    u()
                if DEBUG_DUMPS and b == 0:
                    nc.sync.dma_start(dbg_xa[:], xaT[:])
            for u in pending:
                u()
    nc.compile()
    return nc


_NC = None


def _get_nc():
    global _NC
    if _NC is None:
        _NC = _build()
    return _NC


def _prep_in_maps(inputs):
    f32 = lambda a: np.ascontiguousarray(np.asarray(a), dtype=np.float32)
    xq = f32(inputs["input_q"]).reshape(TOK, D)
    xk = f32(inputs["input_k"]).reshape(TOK, D)
    xv = f32(inputs["input_v"]).reshape(TOK, D)

    def tile_xt(x):
        # [TOK, D] -> [16, 128, 4096]: tile t holds tokens [512t, 512t+512),
        # laid out [ki, ko*512 + j] with d = ko*128 + ki.
        xt = x.T.astype(BF16_NP)                    # [D, TOK]
        xt = xt.reshape(8, 128, 16, 512)            # ko ki t j
        xt = xt.transpose(2, 1, 0, 3)               # t ki ko j
        return np.ascontiguousarray(xt.reshape(16, 128, 4096))

    xqT = tile_xt(xq)
    xkT = tile_xt(xk)
    xvT = tile_xt(xv)

    # fold the (linear, dropout p=0) CP adapter into effective weights
    U = f32(inputs["CP_U_W"])              # [D, R]
    V = f32(inputs["CP_V_W"])              # [R, D]
    CPC = f32(inputs["CP_C"])              # [r, r, R]
    CPATT = f32(inputs["CP_attention"])    # [R, 4]
    cpc = np.einsum("xyr,rf->xyf", CPC, CPATT)      # [r, r, 4]
    weffs = {}
    for i, wname in enumerate(("Wq", "Wk", "Wv", "Wproj")):
        weffs[wname] = f32(inputs[wname]) + (U @ cpc[..., i]) @ V

    def tile_w(w_slice):
        # [D, CPB] -> [128, 8, CPB] with d = ko*128 + ki
        wt = w_slice.astype(BF16_NP).reshape(8, 128, CPB)
        return np.ascontiguousarray(wt.transpose(1, 0, 2))

    in_maps = []
    for c in range(NCORES):
        s = slice(c * CPB, (c + 1) * CPB)
        wp = weffs["Wproj"][s, :].astype(BF16_NP)   # [CPB, D]
        in_maps.append({
            "xqT": xqT, "xkT": xkT, "xvT": xvT,
            "weffq": tile_w(weffs["Wq"][:, s]),
            "weffk": tile_w(weffs["Wk"][:, s]),
            "weffv": tile_w(weffs["Wv"][:, s]),
            "weffp": np.ascontiguousarray(wp.reshape(CPB, 2, 512)),
        })
    return in_maps


def run(inputs, trace=False, trace_cores=None):
    nc = _get_nc()
    in_maps = _prep_in_maps(inputs)
    res = run_bass_kernel_spmd(nc, in_maps, list(range(NCORES)),
                               trace=trace, trace_cores=trace_cores)
    acc = res.results[0]["out"].astype(np.float32).copy()
    for c in range(1, NCORES):
        acc += res.results[c]["out"].astype(np.float32)
    acc += np.asarray(inputs["bproj"], dtype=np.float32)[None, :]
    return acc.reshape(B, N, D), res


def kernel(**inputs):
    out, _ = run(inputs, trace=False)
    return out


# revision 28
# speedup vs baseline: 1.1957x; 1.1957x over previous
"""CP-adapter multi-head attention on 8 Trainium2 NeuronCores.

Hardcoded for B=4, N=2048, D=1024, H=16, hd=64, R=r=64 (fp32 in/out).

Sharding: tensor-parallel over heads.  Core c owns heads (2c, 2c+1) =
columns [128c, 128c+128) of the q/k/v projections and rows [128c, 128c+128)
of the output projection; every core streams the full activations (bf16).
Each core emits a partial output [8192, 1024] bf16; the host sums the 8
partials in f32 and adds the bias (the only cross-core reduction).

Kernel design:
- The CP adapter is linear (dropout p=0), so it folds ON HOST into
  effective weights: W_eff = W + U @ cp @ V, cp = einsum(CP_C, CP_attn).
  The device sees only pre-layouted bf16 weight tiles (<0.5% of FLOPs).
- All matmuls run in bf16 (1 cycle/row of the moving operand, f32
  accumulate in PSUM).  Streams and intermediates are bf16, which halves
  HBM traffic vs f32.
- q/k are produced transposed ([cols, tokens], W_eff stationary / X^T
  moving); v is produced transposed then PE-transposed to natural layout
  with ones columns appended for the softmax denominator.
- Attention per (batch, q-quarter): scores computed transposed,
  ST = K^T-block stationary x Q^T moving, two heads row-tiled into one
  PSUM stripe [128 keys, 2x512 q] per key block; one Exp per stripe on
  ScalarE (free scale=1/8); PV accumulates [V|1] stationary over key
  blocks into O' [65, 512] whose row 64 is the softmax denominator.
- Software pipeline with lookahead: ST(kb+1) is issued BEFORE PV(kb) so
  the in-order PE keeps ScalarE fed back-to-back (PV waits on exp(kb);
  without the lookahead it would block ST(kb+1) and starve ScalarE).
- Fill work (next-batch qkv quarters, deferred projection tiles) is
  sliced into ~430ns sub-units and woven between attention units so the
  PE never idles and the HAM clock stays at full speed.
"""

from contextlib import ExitStack

import numpy as np
import ml_dtypes

try:
    import concourse.bass as bass
except ImportError:  # fallback when sitecustomize paths are absent
    import sys
    sys.path.append("/opt/trn_rl_repo")
    import concourse.bass as bass
import concourse.mybir as mybir
from concourse import bacc, tile
from concourse.bass_utils import run_bass_kernel_spmd
from concourse.masks import make_identity

F32 = mybir.dt.float32
BF16 = mybir.dt.bfloat16
AF = mybir.ActivationFunctionType

B, N, D = 4, 2048, 1024
H, HD = 16, 64
R = 64
NCORES = 8
TOK = B * N            # 8192
CPB = D // NCORES      # 128 cols per core
ATT_SCALE = HD ** -0.5

BF16_NP = ml_dtypes.bfloat16
DEBUG_DUMPS = False


def _build():
    nc = bacc.Bacc(None, target_bir_lowering=False, debug=False)

    # ---- external inputs (per-core views prepared on host) ----
    xqT = nc.dram_tensor("xqT", [16, 128, 4096], BF16, kind="ExternalInput")
    xkT = nc.dram_tensor("xkT", [16, 128, 4096], BF16, kind="ExternalInput")
    xvT = nc.dram_tensor("xvT", [16, 128, 4096], BF16, kind="ExternalInput")
    # effective weights, CP adapter folded on host, pre-layouted:
    # w_eff[t] [ki, ko, j]: W_eff[(ko*128+ki), col_c + j]
    weffq_d = nc.dram_tensor("weffq", [128, 8, CPB], BF16, kind="ExternalInput")
    weffk_d = nc.dram_tensor("weffk", [128, 8, CPB], BF16, kind="ExternalInput")
    weffv_d = nc.dram_tensor("weffv", [128, 8, CPB], BF16, kind="ExternalInput")
    # proj rows for this core: [ki, ch, j]: Wp_eff[row_c + ki, ch*512 + j]
    weffp_d = nc.dram_tensor("weffp", [CPB, 2, 512], BF16, kind="ExternalInput")

    out = nc.dram_tensor("out", [TOK, D], BF16, kind="ExternalOutput")
    if DEBUG_DUMPS:
        dbg_q = nc.dram_tensor("dbg_q", [128, 4, 512], BF16, kind="ExternalOutput")
        dbg_k = nc.dram_tensor("dbg_k", [128, 4, 512], BF16, kind="ExternalOutput")
        dbg_vb = nc.dram_tensor("dbg_vb", [128, 16, 130], BF16,
                                kind="ExternalOutput")
        dbg_pt = nc.dram_tensor("dbg_pt", [128, 1024], BF16, kind="ExternalOutput")
        dbg_osb = nc.dram_tensor("dbg_osb", [65, 512], F32, kind="ExternalOutput")
        dbg_xa = nc.dram_tensor("dbg_xa", [128, 4, 512], BF16,
                                kind="ExternalOutput")

    xT3 = {"q": xqT, "k": xkT, "v": xvT}
    weff_dram = {"q": weffq_d, "k": weffk_d, "v": weffv_d}

    with tile.TileContext(nc) as tc:
        with ExitStack() as es:
            const = es.enter_context(tc.tile_pool(name="const", bufs=1))
            weffp_pool = es.enter_context(tc.tile_pool(name="weff", bufs=1))
            xstream = es.enter_context(tc.tile_pool(name="xstream", bufs=16))
            qkvp = es.enter_context(tc.tile_pool(name="qkv", bufs=2))
            ptp = es.enter_context(tc.tile_pool(name="pt", bufs=3))
            normp = es.enter_context(tc.tile_pool(name="norm", bufs=4))
            outst = es.enter_context(tc.tile_pool(name="outst", bufs=3))
            ps_qkv = es.enter_context(
                tc.tile_pool(name="ps_qkv", bufs=2, space="PSUM"))
            ps_st = es.enter_context(
                tc.tile_pool(name="ps_st", bufs=2, space="PSUM"))
            ps_o = es.enter_context(
                tc.tile_pool(name="ps_o", bufs=2, space="PSUM"))

            # ---------- weights + constants ----------
            # weff DMAs are issued lazily (interleaved with the first xs
            # streams in the startup section) so the k-projection chain
            # reaches the first exp as early as possible.
            weff = {}

            def load_weff(t):
                we = weffp_pool.tile([128, 8, CPB], BF16, name=f"weff{t}")
                nc.sync.dma_start(we[:], weff_dram[t][:])
                weff[t] = we
            weff_p = weffp_pool.tile([CPB, 2, 512], BF16)
            identf = const.tile([128, 128], F32)
            make_identity(nc, identf)
            ident = const.tile([128, 128], BF16)
            nc.vector.tensor_copy(ident[:], identf[:])
            onesf = const.tile([128, 1], F32)
            nc.any.memset(onesf[:], 1.0)

            # ---------- qkv fill sub-units ----------
            def qkv_units(b):
                qkvT = {
                    "q": qkvp.tile([128, 4, 512], BF16, name="qT", tag="qT"),
                    "k": qkvp.tile([128, 4, 512], BF16, name="kT", tag="kT"),
                }
                v_b = qkvp.tile([128, 16, 130], BF16, name="v_b", tag="v_b")
                state = (qkvT, v_b)

                def ones_unit():
                    nc.vector.tensor_copy(
                        v_b[:, :, 64:65],
                        onesf[:, None, :].broadcast_to([128, 16, 1]))
                    nc.vector.tensor_copy(
                        v_b[:, :, 129:130],
                        onesf[:, None, :].broadcast_to([128, 16, 1]))

                def mm_sub(t, tb, dp, holder):
                    # quarter of a projection tile: one xs DMA + 2 matmuls
                    # (~430ns of PE) so fill work can slot into every
                    # exp-wait gap and the PE clock never ramps down.
                    def f():
                        if dp == 0:
                            holder["ps"] = ps_qkv.tile(
                                [128, 512], F32, name="psqkv", tag="psqkv")
                        ps = holder["ps"]
                        xs = xstream.tile([128, 1024], BF16,
                                          name="xs", tag="xs")
                        nc.sync.dma_start(
                            xs[:],
                            xT3[t][b * 4 + tb, :,
                                   dp * 1024:(dp + 1) * 1024])
                        for j in range(2):
                            db = dp * 2 + j
                            nc.tensor.matmul(
                                ps[:], weff[t][:, db, :],
                                xs[:, j * 512:(j + 1) * 512],
                                start=(db == 0), stop=(db == 7))
                        if dp < 3:
                            return
                        if t == "v":
                            vt = outst.tile([128, 512], BF16, name="vt",
                                            tag="vt")
                            nc.vector.tensor_copy(vt[:], ps[:])
                            for j in range(4):
                                kb = tb * 4 + j
                                tp = ps_qkv.tile([128, 128], BF16,
                                                 name="pstr", tag="psqkv")
                                nc.tensor.transpose(
                                    tp[:], vt[:, j * 128:(j + 1) * 128],
                                    ident[:])
                                nc.vector.tensor_copy(v_b[:, kb, 0:64],
                                                      tp[:, 0:64])
                                nc.vector.tensor_copy(v_b[:, kb, 65:129],
                                                      tp[:, 64:128])
                        else:
                            nc.vector.tensor_copy(qkvT[t][:, tb, :], ps[:])
                    return f

                units = [ones_unit]
                for t in ("k", "q", "v"):
                    for tb in range(4):
                        holder = {}
                        for dp in range(4):
                            units.append(mm_sub(t, tb, dp, holder))
                return state, units

            # ---------- attention ----------
            def attn_units(b, state):
                qkvT, v_b = state
                xaT = qkvp.tile([128, 4, 512], BF16, name="xaT", tag="xaT")
                sts, pvs, norms = [], [], []

                def group(qq):
                    o_ps = [
                        ps_o.tile([65, 512], F32, name="o_ps", tag="o_ps")
                        for _ in range(2)
                    ]
                    pt_holder = {}

                    def st_unit(kb):
                        def f():
                            st = ps_st.tile([128, 1024], F32, name="st",
                                            tag="st")
                            ktb, ksub = kb // 4, kb % 4
                            for hh in range(2):
                                ro = hh * 64
                                nc.tensor.matmul(
                                    st[:, hh * 512:(hh + 1) * 512],
                                    qkvT["k"][ro:ro + 64, ktb,
                                              ksub * 128:(ksub + 1) * 128],
                                    qkvT["q"][ro:ro + 64, qq, :],
                                    start=True, stop=True)
                            pt = ptp.tile([128, 1024], BF16, name="pt",
                                          tag="pt")
                            nc.scalar.activation(pt[:], st[:], AF.Exp,
                                                 scale=ATT_SCALE)
                            pt_holder[kb] = pt
                        return f

                    def pv_unit(kb):
                        def f():
                            pt = pt_holder[kb]
                            for hh in range(2):
                                nc.tensor.matmul(
                                    o_ps[hh][:],
                                    v_b[:, kb, hh * 65:(hh + 1) * 65],
                                    pt[:, hh * 512:(hh + 1) * 512],
                                    start=(kb == 0), stop=(kb == 15))
                        return f

                    def norm_unit():
                        # Stage O' (and both denominators) out of PSUM right
                        # away so o_ps frees fast and the next group's PV can
                        # start; one reciprocal covers both heads
                        # (denominators at partitions 0 and 32 -- partition
                        # starts must be 32-aligned).
                        o_sb = [None, None]
                        for hh in range(2):
                            o_sb[hh] = normp.tile([64, 512], F32,
                                                  name=f"o_sb{hh}",
                                                  tag=f"o_sb{hh}")
                            nc.vector.tensor_copy(o_sb[hh][:],
                                                  o_ps[hh][0:64, :])
                        d2 = normp.tile([64, 512], F32, name="d2", tag="d2")
                        nc.gpsimd.memset(d2[:], 1.0)
                        nc.vector.tensor_copy(d2[0:1, :], o_ps[0][64:65, :])
                        nc.vector.tensor_copy(d2[32:33, :], o_ps[1][64:65, :])
                        rec2 = normp.tile([64, 512], F32, name="rec2",
                                          tag="rec2")
                        nc.vector.reciprocal(rec2[:], d2[:])
                        rec_b = normp.tile([1, 512], F32, name="rec_b",
                                           tag="rec_b")
                        nc.vector.tensor_copy(rec_b[:], rec2[32:33, :])
                        for hh in range(2):
                            ro = hh * 64
                            rec64 = normp.tile([64, 512], F32, name="rec64",
                                               tag="rec64")
                            nc.gpsimd.partition_broadcast(
                                rec64[:],
                                rec2[0:1, :] if hh == 0 else rec_b[:])
                            nc.vector.tensor_mul(xaT[ro:ro + 64, qq, :],
                                                 o_sb[hh][:], rec64[:])

                    sts.extend(st_unit(kb) for kb in range(16))
                    pvs.extend(pv_unit(kb) for kb in range(16))
                    norms.append(norm_unit)

                for qq in range(4):
                    group(qq)
                return xaT, sts, pvs, norms

            def proj_units(b, xaT, qq):
                tok0 = b * N

                def tb_unit(tb):
                    def f():
                        sub = tb % 4
                        lx = xaT[:, qq, sub * 128:(sub + 1) * 128]
                        ob = outst.tile([128, 1024], BF16, name="ob", tag="ob")
                        for ch in range(2):
                            ps = ps_qkv.tile([128, 512], F32, name="pspj",
                                             tag="psqkv")
                            nc.tensor.matmul(ps[:], lx, weff_p[:, ch, :],
                                             start=True, stop=True)
                            nc.vector.tensor_copy(
                                ob[:, ch * 512:(ch + 1) * 512], ps[:])
                        nc.sync.dma_start(
                            out[tok0 + tb * 128:tok0 + (tb + 1) * 128, :],
                            ob[:])
                    return f
                return [tb_unit(qq * 4 + j) for j in range(4)]

            def batch_stream(sts, pvs, norms):
                # Global lookahead order: S0 S1 [P0 S2] [P1 S3] ... with
                # norm(qq) right after P(16qq+15).  ST(i+2) immediately
                # follows PV(i), so the in-order PE never lets ScalarE
                # starve -- including across group boundaries.
                su = [("S", sts[0]), ("S", sts[1])]
                for i in range(64):
                    if i + 2 < 64:
                        su.append(("S", sts[i + 2]))
                    su.append(("P", pvs[i]))
                    if i % 16 == 15:
                        su.append(("N", norms[i // 16]))
                return su

            # ---------- batch 0 startup ----------
            state, units0 = qkv_units(0)
            # units0 layout: [ones, k(16 subs), q(16 subs), v(16 subs)]
            ones_u = units0[0]
            k_u = lambda tb: units0[1 + 4 * tb:5 + 4 * tb]
            q_u = lambda tb: units0[17 + 4 * tb:21 + 4 * tb]
            v_u = lambda tb: units0[33 + 4 * tb:37 + 4 * tb]
            ones_u()
            load_weff("k")
            for u in k_u(0):
                u()
            load_weff("q")
            for u in q_u(0):
                u()
            load_weff("v")
            for u in v_u(0):
                u()
            nc.sync.dma_start(weff_p[:], weffp_d[:])

            # ---------- main pipeline ----------
            # proj units of group g are deferred into the unit stream of the
            # NEXT group, so the PE never waits on the normalization chain
            # at a group boundary.
            pending = []
            for b in range(B):
                xaT, sts, pvs, norms = attn_units(b, state)
                fill = []
                if b + 1 < B:
                    state, fill = qkv_units(b + 1)
                fi = 0
                stream = batch_stream(sts, pvs, norms)
                sub_q = []
                p_cnt = 0
                n_cnt = 0
                for si, (kind, u) in enumerate(stream):
                    if b == 0 and si in (0, 6, 14):
                        tbn = {0: 1, 6: 2, 14: 3}[si]
                        sub_q += k_u(tbn) + v_u(tbn) + q_u(tbn)
                    u()
                    if sub_q:
                        sub_q.pop(0)()
                    if kind == "P":
                        pi = p_cnt % 16
                        p_cnt += 1
                        if pi in (5, 7, 9, 11) and pending:
                            pending.pop(0)()
                        elif pi >= 14:
                            pass  # keep DVE clear for the o_ps release
                        elif sub_q:
                            sub_q.pop(0)()
                        elif fi < len(fill):
                            fill[fi]()
                            fi += 1
                    elif kind == "N":
                        pending += proj_units(b, xaT, n_cnt)
                        n_cnt += 1
                        if fi < len(fill):
                            fill[fi]()
                            fi += 1
                for u in sub_q:
                    u()
                for u in fill[fi:]:
                    u()
                if DEBUG_DUMPS and b == 0:
                    nc.sync.dma_start(dbg_xa[:], xaT[:])
            for u in pending:
                u()
    nc.compile()
    return nc


_NC = None


def _get_nc():
    global _NC
    if _NC is None:
        _NC = _build()
    return _NC


def _prep_in_maps(inputs):
    f32 = lambda a: np.ascontiguousarray(np.asarray(a), dtype=np.float32)
    xq = f32(inputs["input_q"]).reshape(TOK, D)
    xk = f32(inputs["input_k"]).reshape(TOK, D)
    xv = f32(inputs["input_v"]).reshape(TOK, D)

    def tile_xt(x):
        # [TOK, D] -> [16, 128, 4096]: tile t holds tokens [512t, 512t+512),
        # laid out [ki, ko*512 + j] with d = ko*128 + ki.
        xt = x.T.astype(BF16_NP)                    # [D, TOK]
        xt = xt.reshape(8, 128, 16, 512)            # ko ki t j
        xt = xt.transpose(2, 1, 0, 3)               # t ki ko j
        return np.ascontiguousarray(xt.reshape(16, 128, 4096))

    xqT = tile_xt(xq)
    xkT = tile_xt(xk)
    xvT = tile_xt(xv)

    # fold the (linear, dropout p=0) CP adapter into effective weights
    U = f32(inputs["CP_U_W"])              # [D, R]
    V = f32(inputs["CP_V_W"])              # [R, D]
    CPC = f32(inputs["CP_C"])              # [r, r, R]
    CPATT = f32(inputs["CP_attention"])    # [R, 4]
    cpc = np.einsum("xyr,rf->xyf", CPC, CPATT)      # [r, r, 4]
    weffs = {}
    for i, wname in enumerate(("Wq", "Wk", "Wv", "Wproj")):
        weffs[wname] = f32(inputs[wname]) + (U @ cpc[..., i]) @ V

    def tile_w(w_slice):
        # [D, CPB] -> [128, 8, CPB] with d = ko*128 + ki
        wt = w_slice.astype(BF16_NP).reshape(8, 128, CPB)
        return np.ascontiguousarray(wt.transpose(1, 0, 2))

    in_maps = []
    for c in range(NCORES):
        s = slice(c * CPB, (c + 1) * CPB)
        wp = weffs["Wproj"][s, :].astype(BF16_NP)   # [CPB, D]
        in_maps.append({
            "xqT": xqT, "xkT": xkT, "xvT": xvT,
            "weffq": tile_w(weffs["Wq"][:, s]),
            "weffk": tile_w(weffs["Wk"][:, s]),
            "weffv": tile_w(weffs["Wv"][:, s]),
            "weffp": np.ascontiguousarray(wp.reshape(CPB, 2, 512)),
        })
    return in_maps


def run(inputs, trace=False, trace_cores=None):
    nc = _get_nc()
    in_maps = _prep_in_maps(inputs)
    res = run_bass_kernel_spmd(nc, in_maps, list(range(NCORES)),
                               trace=trace, trace_cores=trace_cores)
    acc = res.results[0]["out"].astype(np.float32).copy()
    for c in range(1, NCORES):
        acc += res.results[c]["out"].astype(np.float32)
    acc += np.asarray(inputs["bproj"], dtype=np.float32)[None, :]
    return acc.reshape(B, N, D), res


def kernel(**inputs):
    out, _ = run(inputs, trace=False)
    return out


# revision 29
# speedup vs baseline: 1.2020x; 1.0053x over previous
"""CP-adapter multi-head attention on 8 Trainium2 NeuronCores.

Hardcoded for B=4, N=2048, D=1024, H=16, hd=64, R=r=64 (fp32 in/out).

Sharding: tensor-parallel over heads.  Core c owns heads (2c, 2c+1) =
columns [128c, 128c+128) of the q/k/v projections and rows [128c, 128c+128)
of the output projection; every core streams the full activations (bf16).
Each core emits a partial output [8192, 1024] bf16; the host sums the 8
partials in f32 and adds the bias (the only cross-core reduction).

Kernel design:
- The CP adapter is linear (dropout p=0), so it folds ON HOST into
  effective weights: W_eff = W + U @ cp @ V, cp = einsum(CP_C, CP_attn).
  The device sees only pre-layouted bf16 weight tiles (<0.5% of FLOPs).
- All matmuls run in bf16 (1 cycle/row of the moving operand, f32
  accumulate in PSUM).  Streams and intermediates are bf16, which halves
  HBM traffic vs f32.
- q/k are produced transposed ([cols, tokens], W_eff stationary / X^T
  moving); v is produced transposed then PE-transposed to natural layout
  with ones columns appended for the softmax denominator.
- Attention per (batch, q-quarter): scores computed transposed,
  ST = K^T-block stationary x Q^T moving, two heads row-tiled into one
  PSUM stripe [128 keys, 2x512 q] per key block; one Exp per stripe on
  ScalarE (free scale=1/8); PV accumulates [V|1] stationary over key
  blocks into O' [65, 512] whose row 64 is the softmax denominator.
- Software pipeline with lookahead: ST(kb+1) is issued BEFORE PV(kb) so
  the in-order PE keeps ScalarE fed back-to-back (PV waits on exp(kb);
  without the lookahead it would block ST(kb+1) and starve ScalarE).
- Fill work (next-batch qkv quarters, deferred projection tiles) is
  sliced into ~430ns sub-units and woven between attention units so the
  PE never idles and the HAM clock stays at full speed.
"""

from contextlib import ExitStack

import numpy as np
import ml_dtypes

try:
    import concourse.bass as bass
except ImportError:  # fallback when sitecustomize paths are absent
    import sys
    sys.path.append("/opt/trn_rl_repo")
    import concourse.bass as bass
import concourse.mybir as mybir
from concourse import bacc, tile
from concourse.bass_utils import run_bass_kernel_spmd
from concourse.masks import make_identity

F32 = mybir.dt.float32
BF16 = mybir.dt.bfloat16
AF = mybir.ActivationFunctionType

B, N, D = 4, 2048, 1024
H, HD = 16, 64
R = 64
NCORES = 8
TOK = B * N            # 8192
CPB = D // NCORES      # 128 cols per core
ATT_SCALE = HD ** -0.5

BF16_NP = ml_dtypes.bfloat16
DEBUG_DUMPS = False


def _build():
    nc = bacc.Bacc(None, target_bir_lowering=False, debug=False)

    # ---- external inputs (per-core views prepared on host) ----
    xqT = nc.dram_tensor("xqT", [16, 128, 4096], BF16, kind="ExternalInput")
    xkT = nc.dram_tensor("xkT", [16, 128, 4096], BF16, kind="ExternalInput")
    xvT = nc.dram_tensor("xvT", [16, 128, 4096], BF16, kind="ExternalInput")
    # effective weights, CP adapter folded on host, pre-layouted:
    # w_eff[t] [ki, ko, j]: W_eff[(ko*128+ki), col_c + j]
    weffq_d = nc.dram_tensor("weffq", [128, 8, CPB], BF16, kind="ExternalInput")
    weffk_d = nc.dram_tensor("weffk", [128, 8, CPB], BF16, kind="ExternalInput")
    weffv_d = nc.dram_tensor("weffv", [128, 8, CPB], BF16, kind="ExternalInput")
    # proj rows for this core: [ki, ch, j]: Wp_eff[row_c + ki, ch*512 + j]
    weffp_d = nc.dram_tensor("weffp", [CPB, 2, 512], BF16, kind="ExternalInput")

    out = nc.dram_tensor("out", [TOK, D], BF16, kind="ExternalOutput")
    if DEBUG_DUMPS:
        dbg_q = nc.dram_tensor("dbg_q", [128, 4, 512], BF16, kind="ExternalOutput")
        dbg_k = nc.dram_tensor("dbg_k", [128, 4, 512], BF16, kind="ExternalOutput")
        dbg_vb = nc.dram_tensor("dbg_vb", [128, 16, 130], BF16,
                                kind="ExternalOutput")
        dbg_pt = nc.dram_tensor("dbg_pt", [128, 1024], BF16, kind="ExternalOutput")
        dbg_osb = nc.dram_tensor("dbg_osb", [65, 512], F32, kind="ExternalOutput")
        dbg_xa = nc.dram_tensor("dbg_xa", [128, 4, 512], BF16,
                                kind="ExternalOutput")

    xT3 = {"q": xqT, "k": xkT, "v": xvT}
    weff_dram = {"q": weffq_d, "k": weffk_d, "v": weffv_d}

    with tile.TileContext(nc) as tc:
        with ExitStack() as es:
            const = es.enter_context(tc.tile_pool(name="const", bufs=1))
            weffp_pool = es.enter_context(tc.tile_pool(name="weff", bufs=1))
            xstream = es.enter_context(tc.tile_pool(name="xstream", bufs=16))
            qkvp = es.enter_context(tc.tile_pool(name="qkv", bufs=2))
            ptp = es.enter_context(tc.tile_pool(name="pt", bufs=3))
            normp = es.enter_context(tc.tile_pool(name="norm", bufs=6))
            outst = es.enter_context(tc.tile_pool(name="outst", bufs=3))
            ps_qkv = es.enter_context(
                tc.tile_pool(name="ps_qkv", bufs=2, space="PSUM"))
            ps_st = es.enter_context(
                tc.tile_pool(name="ps_st", bufs=2, space="PSUM"))
            ps_o = es.enter_context(
                tc.tile_pool(name="ps_o", bufs=2, space="PSUM"))

            # ---------- weights + constants ----------
            # weff DMAs are issued lazily (interleaved with the first xs
            # streams in the startup section) so the k-projection chain
            # reaches the first exp as early as possible.
            weff = {}

            def load_weff(t):
                we = weffp_pool.tile([128, 8, CPB], BF16, name=f"weff{t}")
                nc.sync.dma_start(we[:], weff_dram[t][:])
                weff[t] = we
            weff_p = weffp_pool.tile([CPB, 2, 512], BF16)
            identf = const.tile([128, 128], F32)
            make_identity(nc, identf)
            ident = const.tile([128, 128], BF16)
            nc.vector.tensor_copy(ident[:], identf[:])
            onesf = const.tile([128, 1], F32)
            nc.any.memset(onesf[:], 1.0)

            # ---------- qkv fill sub-units ----------
            prestage = {}

            def qkv_units(b):
                qkvT = {
                    "q": qkvp.tile([128, 4, 512], BF16, name="qT", tag="qT"),
                    "k": qkvp.tile([128, 4, 512], BF16, name="kT", tag="kT"),
                }
                v_b = qkvp.tile([128, 16, 130], BF16, name="v_b", tag="v_b")
                state = (qkvT, v_b)

                def ones_unit():
                    nc.vector.tensor_copy(
                        v_b[:, :, 64:65],
                        onesf[:, None, :].broadcast_to([128, 16, 1]))
                    nc.vector.tensor_copy(
                        v_b[:, :, 129:130],
                        onesf[:, None, :].broadcast_to([128, 16, 1]))

                def mm_sub(t, tb, dp, holder):
                    # quarter of a projection tile: one xs DMA + 2 matmuls
                    # (~430ns of PE) so fill work can slot into every
                    # exp-wait gap and the PE clock never ramps down.
                    def f():
                        if dp == 0:
                            holder["ps"] = ps_qkv.tile(
                                [128, 512], F32, name="psqkv", tag="psqkv")
                        ps = holder["ps"]
                        xs = prestage.pop((b, t, tb, dp), None)
                        if xs is None:
                            xs = xstream.tile([128, 1024], BF16,
                                              name="xs", tag="xs")
                            nc.sync.dma_start(
                                xs[:],
                                xT3[t][b * 4 + tb, :,
                                       dp * 1024:(dp + 1) * 1024])
                        for j in range(2):
                            db = dp * 2 + j
                            nc.tensor.matmul(
                                ps[:], weff[t][:, db, :],
                                xs[:, j * 512:(j + 1) * 512],
                                start=(db == 0), stop=(db == 7))
                        if dp < 3:
                            return
                        if t == "v":
                            vt = outst.tile([128, 512], BF16, name="vt",
                                            tag="vt")
                            nc.vector.tensor_copy(vt[:], ps[:])
                            for j in range(4):
                                kb = tb * 4 + j
                                tp = ps_qkv.tile([128, 128], BF16,
                                                 name="pstr", tag="psqkv")
                                nc.tensor.transpose(
                                    tp[:], vt[:, j * 128:(j + 1) * 128],
                                    ident[:])
                                nc.vector.tensor_copy(v_b[:, kb, 0:64],
                                                      tp[:, 0:64])
                                nc.vector.tensor_copy(v_b[:, kb, 65:129],
                                                      tp[:, 64:128])
                        else:
                            nc.vector.tensor_copy(qkvT[t][:, tb, :], ps[:])
                    return f

                units = [ones_unit]
                for t in ("k", "q", "v"):
                    for tb in range(4):
                        holder = {}
                        for dp in range(4):
                            units.append(mm_sub(t, tb, dp, holder))
                return state, units

            # ---------- attention ----------
            def attn_units(b, state):
                qkvT, v_b = state
                xaT = qkvp.tile([128, 4, 512], BF16, name="xaT", tag="xaT")
                sts, pvs, norms = [], [], []

                def group(qq):
                    o_ps = [
                        ps_o.tile([65, 512], F32, name="o_ps", tag="o_ps")
                        for _ in range(2)
                    ]
                    pt_holder = {}

                    def st_unit(kb):
                        def f():
                            st = ps_st.tile([128, 1024], F32, name="st",
                                            tag="st")
                            ktb, ksub = kb // 4, kb % 4
                            for hh in range(2):
                                ro = hh * 64
                                nc.tensor.matmul(
                                    st[:, hh * 512:(hh + 1) * 512],
                                    qkvT["k"][ro:ro + 64, ktb,
                                              ksub * 128:(ksub + 1) * 128],
                                    qkvT["q"][ro:ro + 64, qq, :],
                                    start=True, stop=True)
                            pt = ptp.tile([128, 1024], BF16, name="pt",
                                          tag="pt")
                            nc.scalar.activation(pt[:], st[:], AF.Exp,
                                                 scale=ATT_SCALE)
                            pt_holder[kb] = pt
                        return f

                    def pv_unit(kb):
                        def f():
                            pt = pt_holder[kb]
                            for hh in range(2):
                                nc.tensor.matmul(
                                    o_ps[hh][:],
                                    v_b[:, kb, hh * 65:(hh + 1) * 65],
                                    pt[:, hh * 512:(hh + 1) * 512],
                                    start=(kb == 0), stop=(kb == 15))
                        return f

                    def norm_unit():
                        # Stage O' AND the denominator row out of PSUM in a
                        # single [65,512] copy per head -- o_ps frees after
                        # just two DVE ops, so the next group's first PV
                        # (which reuses the slot, start=True) is not held up
                        # by the rest of the normalization chain.
                        o_sb = [None, None]
                        for hh in range(2):
                            o_sb[hh] = normp.tile([65, 512], F32,
                                                  name=f"o_sb{hh}",
                                                  tag=f"o_sb{hh}")
                            nc.vector.tensor_copy(o_sb[hh][:],
                                                  o_ps[hh][:, :])
                        d2 = normp.tile([64, 512], F32, name="d2", tag="d2")
                        nc.gpsimd.memset(d2[:], 1.0)
                        nc.vector.tensor_copy(d2[0:1, :], o_sb[0][64:65, :])
                        nc.vector.tensor_copy(d2[32:33, :], o_sb[1][64:65, :])
                        rec2 = normp.tile([64, 512], F32, name="rec2",
                                          tag="rec2")
                        nc.vector.reciprocal(rec2[:], d2[:])
                        rec_b = normp.tile([1, 512], F32, name="rec_b",
                                           tag="rec_b")
                        nc.vector.tensor_copy(rec_b[:], rec2[32:33, :])
                        for hh in range(2):
                            ro = hh * 64
                            rec64 = normp.tile([64, 512], F32, name="rec64",
                                               tag="rec64")
                            nc.gpsimd.partition_broadcast(
                                rec64[:],
                                rec2[0:1, :] if hh == 0 else rec_b[:])
                            nc.vector.tensor_mul(xaT[ro:ro + 64, qq, :],
                                                 o_sb[hh][0:64, :], rec64[:])

                    sts.extend(st_unit(kb) for kb in range(16))
                    pvs.extend(pv_unit(kb) for kb in range(16))
                    norms.append(norm_unit)

                for qq in range(4):
                    group(qq)
                return xaT, sts, pvs, norms

            def proj_units(b, xaT, qq):
                tok0 = b * N

                def tb_unit(tb):
                    def f():
                        sub = tb % 4
                        lx = xaT[:, qq, sub * 128:(sub + 1) * 128]
                        ob = outst.tile([128, 1024], BF16, name="ob", tag="ob")
                        for ch in range(2):
                            ps = ps_qkv.tile([128, 512], F32, name="pspj",
                                             tag="psqkv")
                            nc.tensor.matmul(ps[:], lx, weff_p[:, ch, :],
                                             start=True, stop=True)
                            nc.vector.tensor_copy(
                                ob[:, ch * 512:(ch + 1) * 512], ps[:])
                        nc.sync.dma_start(
                            out[tok0 + tb * 128:tok0 + (tb + 1) * 128, :],
                            ob[:])
                    return f
                return [tb_unit(qq * 4 + j) for j in range(4)]

            def batch_stream(sts, pvs, norms):
                # Global lookahead order: S0 S1 [P0 S2] [P1 S3] ... with
                # norm(qq) right after P(16qq+15).  ST(i+2) immediately
                # follows PV(i), so the in-order PE never lets ScalarE
                # starve -- including across group boundaries.
                su = [("S", sts[0]), ("S", sts[1])]
                for i in range(64):
                    if i + 2 < 64:
                        su.append(("S", sts[i + 2]))
                    su.append(("P", pvs[i]))
                    if i % 16 == 15:
                        su.append(("N", norms[i // 16]))
                return su

            # ---------- batch 0 startup ----------
            state, units0 = qkv_units(0)
            # units0 layout: [ones, k(16 subs), q(16 subs), v(16 subs)]
            ones_u = units0[0]
            k_u = lambda tb: units0[1 + 4 * tb:5 + 4 * tb]
            q_u = lambda tb: units0[17 + 4 * tb:21 + 4 * tb]
            v_u = lambda tb: units0[33 + 4 * tb:37 + 4 * tb]
            # kick the critical DMAs first: weffk, then the 12 tb0
            # streams, then the remaining weights
            load_weff("k")
            for t in ("k", "q", "v"):
                for dp in range(4):
                    xs = xstream.tile([128, 1024], BF16, name="xs", tag="xs")
                    nc.sync.dma_start(xs[:],
                                      xT3[t][0, :, dp * 1024:(dp + 1) * 1024])
                    prestage[(0, t, 0, dp)] = xs
            load_weff("q")
            load_weff("v")
            ones_u()
            for u in k_u(0) + q_u(0) + v_u(0):
                u()
            nc.sync.dma_start(weff_p[:], weffp_d[:])

            # ---------- main pipeline ----------
            # proj units of group g are deferred into the unit stream of the
            # NEXT group, so the PE never waits on the normalization chain
            # at a group boundary.
            pending = []
            for b in range(B):
                xaT, sts, pvs, norms = attn_units(b, state)
                fill = []
                if b + 1 < B:
                    state, fill = qkv_units(b + 1)
                fi = 0
                stream = batch_stream(sts, pvs, norms)
                sub_q = []
                p_cnt = 0
                n_cnt = 0
                for si, (kind, u) in enumerate(stream):
                    if b == 0 and si in (0, 6, 14):
                        tbn = {0: 1, 6: 2, 14: 3}[si]
                        sub_q += k_u(tbn) + v_u(tbn) + q_u(tbn)
                    u()
                    if sub_q:
                        sub_q.pop(0)()
                    if kind == "P":
                        pi = p_cnt % 16
                        p_cnt += 1
                        if pi in (5, 7, 9, 11) and pending:
                            pending.pop(0)()
                        elif pi >= 14:
                            pass  # keep DVE clear for the o_ps release
                        elif sub_q:
                            sub_q.pop(0)()
                        elif fi < len(fill):
                            fill[fi]()
                            fi += 1
                    elif kind == "N":
                        pending += proj_units(b, xaT, n_cnt)
                        n_cnt += 1
                        if fi < len(fill):
                            fill[fi]()
                            fi += 1
                for u in sub_q:
                    u()
                for u in fill[fi:]:
                    u()
                if DEBUG_DUMPS and b == 0:
                    nc.sync.dma_start(dbg_xa[:], xaT[:])
            for u in pending:
                u()
    nc.compile()
    return nc


_NC = None


def _get_nc():
    global _NC
    if _NC is None:
        _NC = _build()
    return _NC


def _prep_in_maps(inputs):
    f32 = lambda a: np.ascontiguousarray(np.asarray(a), dtype=np.float32)
    xq = f32(inputs["input_q"]).reshape(TOK, D)
    xk = f32(inputs["input_k"]).reshape(TOK, D)
    xv = f32(inputs["input_v"]).reshape(TOK, D)

    def tile_xt(x):
        # [TOK, D] -> [16, 128, 4096]: tile t holds tokens [512t, 512t+512),
        # laid out [ki, ko*512 + j] with d = ko*128 + ki.
        xt = x.T.astype(BF16_NP)                    # [D, TOK]
        xt = xt.reshape(8, 128, 16, 512)            # ko ki t j
        xt = xt.transpose(2, 1, 0, 3)               # t ki ko j
        return np.ascontiguousarray(xt.reshape(16, 128, 4096))

    xqT = tile_xt(xq)
    xkT = tile_xt(xk)
    xvT = tile_xt(xv)

    # fold the (linear, dropout p=0) CP adapter into effective weights
    U = f32(inputs["CP_U_W"])              # [D, R]
    V = f32(inputs["CP_V_W"])              # [R, D]
    CPC = f32(inputs["CP_C"])              # [r, r, R]
    CPATT = f32(inputs["CP_attention"])    # [R, 4]
    cpc = np.einsum("xyr,rf->xyf", CPC, CPATT)      # [r, r, 4]
    weffs = {}
    for i, wname in enumerate(("Wq", "Wk", "Wv", "Wproj")):
        weffs[wname] = f32(inputs[wname]) + (U @ cpc[..., i]) @ V

    def tile_w(w_slice):
        # [D, CPB] -> [128, 8, CPB] with d = ko*128 + ki
        wt = w_slice.astype(BF16_NP).reshape(8, 128, CPB)
        return np.ascontiguousarray(wt.transpose(1, 0, 2))

    in_maps = []
    for c in range(NCORES):
        s = slice(c * CPB, (c + 1) * CPB)
        wp = weffs["Wproj"][s, :].astype(BF16_NP)   # [CPB, D]
        in_maps.append({
            "xqT": xqT, "xkT": xkT, "xvT": xvT,
            "weffq": tile_w(weffs["Wq"][:, s]),
            "weffk": tile_w(weffs["Wk"][:, s]),
            "weffv": tile_w(weffs["Wv"][:, s]),
            "weffp": np.ascontiguousarray(wp.reshape(CPB, 2, 512)),
        })
    return in_maps


def run(inputs, trace=False, trace_cores=None):
    nc = _get_nc()
    in_maps = _prep_in_maps(inputs)
    res = run_bass_kernel_spmd(nc, in_maps, list(range(NCORES)),
                               trace=trace, trace_cores=trace_cores)
    acc = res.results[0]["out"].astype(np.float32).copy()
    for c in range(1, NCORES):
        acc += res.results[c]["out"].astype(np.float32)
    acc += np.asarray(inputs["bproj"], dtype=np.float32)[None, :]
    return acc.reshape(B, N, D), res


def kernel(**inputs):
    out, _ = run(inputs, trace=False)
    return out


# revision 30
# speedup vs baseline: 1.2048x; 1.0023x over previous
"""CP-adapter multi-head attention on 8 Trainium2 NeuronCores.

Hardcoded for B=4, N=2048, D=1024, H=16, hd=64, R=r=64 (fp32 in/out).

Sharding: tensor-parallel over heads.  Core c owns heads (2c, 2c+1) =
columns [128c, 128c+128) of the q/k/v projections and rows [128c, 128c+128)
of the output projection; every core streams the full activations (bf16).
Each core emits a partial output [8192, 1024] bf16; the host sums the 8
partials in f32 and adds the bias (the only cross-core reduction).

Kernel design:
- The CP adapter is linear (dropout p=0), so it folds ON HOST into
  effective weights: W_eff = W + U @ cp @ V, cp = einsum(CP_C, CP_attn).
  The device sees only pre-layouted bf16 weight tiles (<0.5% of FLOPs).
- All matmuls run in bf16 (1 cycle/row of the moving operand, f32
  accumulate in PSUM).  Streams and intermediates are bf16, which halves
  HBM traffic vs f32.
- q/k are produced transposed ([cols, tokens], W_eff stationary / X^T
  moving); v is produced transposed then PE-transposed to natural layout
  with ones columns appended for the softmax denominator.
- Attention per (batch, q-quarter): scores computed transposed,
  ST = K^T-block stationary x Q^T moving, two heads row-tiled into one
  PSUM stripe [128 keys, 2x512 q] per key block; one Exp per stripe on
  ScalarE (free scale=1/8); PV accumulates [V|1] stationary over key
  blocks into O' [65, 512] whose row 64 is the softmax denominator.
- Software pipeline with lookahead: ST(kb+1) is issued BEFORE PV(kb) so
  the in-order PE keeps ScalarE fed back-to-back (PV waits on exp(kb);
  without the lookahead it would block ST(kb+1) and starve ScalarE).
- Fill work (next-batch qkv quarters, deferred projection tiles) is
  sliced into ~430ns sub-units and woven between attention units so the
  PE never idles and the HAM clock stays at full speed.
"""

from contextlib import ExitStack

import numpy as np
import ml_dtypes

try:
    import concourse.bass as bass
except ImportError:  # fallback when sitecustomize paths are absent
    import sys
    sys.path.append("/opt/trn_rl_repo")
    import concourse.bass as bass
import concourse.mybir as mybir
from concourse import bacc, tile
from concourse.bass_utils import run_bass_kernel_spmd
from concourse.masks import make_identity

F32 = mybir.dt.float32
BF16 = mybir.dt.bfloat16
AF = mybir.ActivationFunctionType

B, N, D = 4, 2048, 1024
H, HD = 16, 64
R = 64
NCORES = 8
TOK = B * N            # 8192
CPB = D // NCORES      # 128 cols per core
ATT_SCALE = HD ** -0.5

BF16_NP = ml_dtypes.bfloat16
DEBUG_DUMPS = False


def _build():
    nc = bacc.Bacc(None, target_bir_lowering=False, debug=False)

    # ---- external inputs (per-core views prepared on host) ----
    xqT = nc.dram_tensor("xqT", [16, 128, 4096], BF16, kind="ExternalInput")
    xkT = nc.dram_tensor("xkT", [16, 128, 4096], BF16, kind="ExternalInput")
    xvT = nc.dram_tensor("xvT", [16, 128, 4096], BF16, kind="ExternalInput")
    # effective weights, CP adapter folded on host, pre-layouted:
    # w_eff[t] [ki, ko, j]: W_eff[(ko*128+ki), col_c + j]
    weffq_d = nc.dram_tensor("weffq", [128, 8, CPB], BF16, kind="ExternalInput")
    weffk_d = nc.dram_tensor("weffk", [128, 8, CPB], BF16, kind="ExternalInput")
    weffv_d = nc.dram_tensor("weffv", [128, 8, CPB], BF16, kind="ExternalInput")
    # proj rows for this core: [ki, ch, j]: Wp_eff[row_c + ki, ch*512 + j]
    weffp_d = nc.dram_tensor("weffp", [CPB, 2, 512], BF16, kind="ExternalInput")

    out = nc.dram_tensor("out", [TOK, D], BF16, kind="ExternalOutput")
    if DEBUG_DUMPS:
        dbg_q = nc.dram_tensor("dbg_q", [128, 4, 512], BF16, kind="ExternalOutput")
        dbg_k = nc.dram_tensor("dbg_k", [128, 4, 512], BF16, kind="ExternalOutput")
        dbg_vb = nc.dram_tensor("dbg_vb", [128, 16, 130], BF16,
                                kind="ExternalOutput")
        dbg_pt = nc.dram_tensor("dbg_pt", [128, 1024], BF16, kind="ExternalOutput")
        dbg_osb = nc.dram_tensor("dbg_osb", [65, 512], F32, kind="ExternalOutput")
        dbg_xa = nc.dram_tensor("dbg_xa", [128, 4, 512], BF16,
                                kind="ExternalOutput")

    xT3 = {"q": xqT, "k": xkT, "v": xvT}
    weff_dram = {"q": weffq_d, "k": weffk_d, "v": weffv_d}

    with tile.TileContext(nc) as tc:
        with ExitStack() as es:
            const = es.enter_context(tc.tile_pool(name="const", bufs=1))
            weffp_pool = es.enter_context(tc.tile_pool(name="weff", bufs=1))
            xstream = es.enter_context(tc.tile_pool(name="xstream", bufs=16))
            qkvp = es.enter_context(tc.tile_pool(name="qkv", bufs=2))
            ptp = es.enter_context(tc.tile_pool(name="pt", bufs=3))
            normp = es.enter_context(tc.tile_pool(name="norm", bufs=6))
            outst = es.enter_context(tc.tile_pool(name="outst", bufs=3))
            ps_qkv = es.enter_context(
                tc.tile_pool(name="ps_qkv", bufs=2, space="PSUM"))
            ps_st = es.enter_context(
                tc.tile_pool(name="ps_st", bufs=2, space="PSUM"))
            ps_o = es.enter_context(
                tc.tile_pool(name="ps_o", bufs=2, space="PSUM"))

            # ---------- weights + constants ----------
            # weff DMAs are issued lazily (interleaved with the first xs
            # streams in the startup section) so the k-projection chain
            # reaches the first exp as early as possible.
            weff = {}

            def load_weff(t):
                we = weffp_pool.tile([128, 8, CPB], BF16, name=f"weff{t}")
                nc.sync.dma_start(we[:], weff_dram[t][:])
                weff[t] = we
            weff_p = weffp_pool.tile([CPB, 2, 512], BF16)
            identf = const.tile([128, 128], F32)
            make_identity(nc, identf)
            ident = const.tile([128, 128], BF16)
            nc.vector.tensor_copy(ident[:], identf[:])
            onesf = const.tile([128, 1], F32)
            nc.any.memset(onesf[:], 1.0)

            # ---------- qkv fill sub-units ----------
            prestage = {}

            def qkv_units(b):
                qkvT = {
                    "q": qkvp.tile([128, 4, 512], BF16, name="qT", tag="qT"),
                    "k": qkvp.tile([128, 4, 512], BF16, name="kT", tag="kT"),
                }
                v_b = qkvp.tile([128, 16, 130], BF16, name="v_b", tag="v_b")
                state = (qkvT, v_b)

                def ones_unit():
                    nc.vector.tensor_copy(
                        v_b[:, :, 64:65],
                        onesf[:, None, :].broadcast_to([128, 16, 1]))
                    nc.vector.tensor_copy(
                        v_b[:, :, 129:130],
                        onesf[:, None, :].broadcast_to([128, 16, 1]))

                def mm_sub(t, tb, dp, holder):
                    # quarter of a projection tile: one xs DMA + 2 matmuls
                    # (~430ns of PE) so fill work can slot into every
                    # exp-wait gap and the PE clock never ramps down.
                    def f():
                        if dp == 0:
                            holder["ps"] = ps_qkv.tile(
                                [128, 512], F32, name="psqkv", tag="psqkv")
                        ps = holder["ps"]
                        xs = prestage.pop((b, t, tb, dp), None)
                        if xs is None:
                            xs = xstream.tile([128, 1024], BF16,
                                              name="xs", tag="xs")
                            nc.sync.dma_start(
                                xs[:],
                                xT3[t][b * 4 + tb, :,
                                       dp * 1024:(dp + 1) * 1024])
                        for j in range(2):
                            db = dp * 2 + j
                            nc.tensor.matmul(
                                ps[:], weff[t][:, db, :],
                                xs[:, j * 512:(j + 1) * 512],
                                start=(db == 0), stop=(db == 7))
                        if dp < 3:
                            return
                        if t == "v":
                            vt = outst.tile([128, 512], BF16, name="vt",
                                            tag="vt")
                            nc.vector.tensor_copy(vt[:], ps[:])
                            for j in range(4):
                                kb = tb * 4 + j
                                tp = ps_qkv.tile([128, 128], BF16,
                                                 name="pstr", tag="psqkv")
                                nc.tensor.transpose(
                                    tp[:], vt[:, j * 128:(j + 1) * 128],
                                    ident[:])
                                nc.vector.tensor_copy(v_b[:, kb, 0:64],
                                                      tp[:, 0:64])
                                nc.vector.tensor_copy(v_b[:, kb, 65:129],
                                                      tp[:, 64:128])
                        else:
                            nc.vector.tensor_copy(qkvT[t][:, tb, :], ps[:])
                    return f

                units = [ones_unit]
                for t in ("k", "q", "v"):
                    for tb in range(4):
                        holder = {}
                        for dp in range(4):
                            units.append(mm_sub(t, tb, dp, holder))
                return state, units

            # ---------- attention ----------
            def attn_units(b, state):
                qkvT, v_b = state
                xaT = qkvp.tile([128, 4, 512], BF16, name="xaT", tag="xaT")
                sts, pvs, norms = [], [], []

                def group(qq):
                    o_ps = [
                        ps_o.tile([65, 512], F32, name="o_ps", tag="o_ps")
                        for _ in range(2)
                    ]
                    pt_holder = {}

                    def st_unit(kb):
                        def f():
                            st = ps_st.tile([128, 1024], F32, name="st",
                                            tag="st")
                            ktb, ksub = kb // 4, kb % 4
                            for hh in range(2):
                                ro = hh * 64
                                nc.tensor.matmul(
                                    st[:, hh * 512:(hh + 1) * 512],
                                    qkvT["k"][ro:ro + 64, ktb,
                                              ksub * 128:(ksub + 1) * 128],
                                    qkvT["q"][ro:ro + 64, qq, :],
                                    start=True, stop=True)
                            pt = ptp.tile([128, 1024], BF16, name="pt",
                                          tag="pt")
                            nc.scalar.activation(pt[:], st[:], AF.Exp,
                                                 scale=ATT_SCALE)
                            pt_holder[kb] = pt
                        return f

                    def pv_unit(kb):
                        def f():
                            pt = pt_holder[kb]
                            for hh in range(2):
                                nc.tensor.matmul(
                                    o_ps[hh][:],
                                    v_b[:, kb, hh * 65:(hh + 1) * 65],
                                    pt[:, hh * 512:(hh + 1) * 512],
                                    start=(kb == 0), stop=(kb == 15))
                        return f

                    def norm_unit():
                        # Stage O' AND the denominator row out of PSUM in a
                        # single [65,512] copy per head -- o_ps frees after
                        # just two DVE ops, so the next group's first PV
                        # (which reuses the slot, start=True) is not held up
                        # by the rest of the normalization chain.
                        o_sb = [None, None]
                        for hh in range(2):
                            o_sb[hh] = normp.tile([65, 512], F32,
                                                  name=f"o_sb{hh}",
                                                  tag=f"o_sb{hh}")
                            nc.vector.tensor_copy(o_sb[hh][:],
                                                  o_ps[hh][:, :])
                        d2 = normp.tile([64, 512], F32, name="d2", tag="d2")
                        nc.gpsimd.memset(d2[:], 1.0)
                        nc.vector.tensor_copy(d2[0:1, :], o_sb[0][64:65, :])
                        nc.vector.tensor_copy(d2[32:33, :], o_sb[1][64:65, :])
                        rec2 = normp.tile([64, 512], F32, name="rec2",
                                          tag="rec2")
                        nc.vector.reciprocal(rec2[:], d2[:])
                        rec_b = normp.tile([1, 512], F32, name="rec_b",
                                           tag="rec_b")
                        nc.vector.tensor_copy(rec_b[:], rec2[32:33, :])
                        for hh in range(2):
                            ro = hh * 64
                            rec64 = normp.tile([64, 512], F32, name="rec64",
                                               tag="rec64")
                            nc.gpsimd.partition_broadcast(
                                rec64[:],
                                rec2[0:1, :] if hh == 0 else rec_b[:])
                            nc.vector.tensor_mul(xaT[ro:ro + 64, qq, :],
                                                 o_sb[hh][0:64, :], rec64[:])

                    sts.extend(st_unit(kb) for kb in range(16))
                    pvs.extend(pv_unit(kb) for kb in range(16))
                    norms.append(norm_unit)

                for qq in range(4):
                    group(qq)
                return xaT, sts, pvs, norms

            def proj_units(b, xaT, qq):
                tok0 = b * N

                def tb_unit(tb):
                    def f():
                        sub = tb % 4
                        lx = xaT[:, qq, sub * 128:(sub + 1) * 128]
                        ob = outst.tile([128, 1024], BF16, name="ob", tag="ob")
                        for ch in range(2):
                            ps = ps_qkv.tile([128, 512], F32, name="pspj",
                                             tag="psqkv")
                            nc.tensor.matmul(ps[:], lx, weff_p[:, ch, :],
                                             start=True, stop=True)
                            nc.vector.tensor_copy(
                                ob[:, ch * 512:(ch + 1) * 512], ps[:])
                        nc.sync.dma_start(
                            out[tok0 + tb * 128:tok0 + (tb + 1) * 128, :],
                            ob[:])
                    return f
                return [tb_unit(qq * 4 + j) for j in range(4)]

            def batch_stream(sts, pvs, norms):
                # Global lookahead order: S0 S1 [P0 S2] [P1 S3] ... with
                # norm(qq) right after P(16qq+15).  ST(i+2) immediately
                # follows PV(i), so the in-order PE never lets ScalarE
                # starve -- including across group boundaries.
                su = [("S", sts[0]), ("S", sts[1])]
                for i in range(64):
                    if i + 2 < 64:
                        su.append(("S", sts[i + 2]))
                    su.append(("P", pvs[i]))
                    if i % 16 == 15:
                        su.append(("N", norms[i // 16]))
                return su

            # ---------- batch 0 startup ----------
            state, units0 = qkv_units(0)
            # units0 layout: [ones, k(16 subs), q(16 subs), v(16 subs)]
            ones_u = units0[0]
            k_u = lambda tb: units0[1 + 4 * tb:5 + 4 * tb]
            q_u = lambda tb: units0[17 + 4 * tb:21 + 4 * tb]
            v_u = lambda tb: units0[33 + 4 * tb:37 + 4 * tb]
            # kick the critical DMAs first: weffk, then the 12 tb0
            # streams, then the remaining weights
            load_weff("k")
            for t in ("k", "q", "v"):
                for dp in range(4):
                    xs = xstream.tile([128, 1024], BF16, name="xs", tag="xs")
                    nc.sync.dma_start(xs[:],
                                      xT3[t][0, :, dp * 1024:(dp + 1) * 1024])
                    prestage[(0, t, 0, dp)] = xs
            load_weff("q")
            load_weff("v")
            ones_u()
            for u in k_u(0) + q_u(0) + v_u(0):
                u()
            nc.sync.dma_start(weff_p[:], weffp_d[:])

            # ---------- main pipeline ----------
            # proj units of group g are deferred into the unit stream of the
            # NEXT group, so the PE never waits on the normalization chain
            # at a group boundary.
            pending = []
            for b in range(B):
                xaT, sts, pvs, norms = attn_units(b, state)
                fill = []
                if b + 1 < B:
                    state, fill = qkv_units(b + 1)
                fi = 0
                stream = batch_stream(sts, pvs, norms)
                sub_q = []
                p_cnt = 0
                n_cnt = 0
                for si, (kind, u) in enumerate(stream):
                    if b == 0 and si in (0, 6, 14):
                        tbn = {0: 1, 6: 2, 14: 3}[si]
                        sub_q += k_u(tbn) + v_u(tbn) + q_u(tbn)
                    u()
                    if sub_q:
                        sub_q.pop(0)()
                    if kind == "P":
                        pi = p_cnt % 16
                        p_cnt += 1
                        # last batch has no qkv fill: hold proj backlog and
                        # spend it at group boundaries (pi 0) and the tail,
                        # where the PE would otherwise idle on the norm chain
                        psl = (5, 7, 9, 11) if fill else (0, 5, 9)
                        if pi in psl and pending:
                            pending.pop(0)()
                        elif pi >= 14:
                            pass  # keep DVE clear for the o_ps release
                        elif sub_q:
                            sub_q.pop(0)()
                        elif fi < len(fill):
                            fill[fi]()
                            fi += 1
                    elif kind == "N":
                        pending += proj_units(b, xaT, n_cnt)
                        n_cnt += 1
                        if fi < len(fill):
                            fill[fi]()
                            fi += 1
                for u in sub_q:
                    u()
                for u in fill[fi:]:
                    u()
                if DEBUG_DUMPS and b == 0:
                    nc.sync.dma_start(dbg_xa[:], xaT[:])
            for u in pending:
                u()
    nc.compile()
    return nc


_NC = None


def _get_nc():
    global _NC
    if _NC is None:
        _NC = _build()
    return _NC


def _prep_in_maps(inputs):
    f32 = lambda a: np.ascontiguousarray(np.asarray(a), dtype=np.float32)
    xq = f32(inputs["input_q"]).reshape(TOK, D)
    xk = f32(inputs["input_k"]).reshape(TOK, D)
    xv = f32(inputs["input_v"]).reshape(TOK, D)

    def tile_xt(x):
        # [TOK, D] -> [16, 128, 4096]: tile t holds tokens [512t, 512t+512),
        # laid out [ki, ko*512 + j] with d = ko*128 + ki.
        xt = x.T.astype(BF16_NP)                    # [D, TOK]
        xt = xt.reshape(8, 128, 16, 512)            # ko ki t j
        xt = xt.transpose(2, 1, 0, 3)               # t ki ko j
        return np.ascontiguousarray(xt.reshape(16, 128, 4096))

    xqT = tile_xt(xq)
    xkT = tile_xt(xk)
    xvT = tile_xt(xv)

    # fold the (linear, dropout p=0) CP adapter into effective weights
    U = f32(inputs["CP_U_W"])              # [D, R]
    V = f32(inputs["CP_V_W"])              # [R, D]
    CPC = f32(inputs["CP_C"])              # [r, r, R]
    CPATT = f32(inputs["CP_attention"])    # [R, 4]
    cpc = np.einsum("xyr,rf->xyf", CPC, CPATT)      # [r, r, 4]
    weffs = {}
    for i, wname in enumerate(("Wq", "Wk", "Wv", "Wproj")):
        weffs[wname] = f32(inputs[wname]) + (U @ cpc[..., i]) @ V

    def tile_w(w_slice):
        # [D, CPB] -> [128, 8, CPB] with d = ko*128 + ki
        wt = w_slice.astype(BF16_NP).reshape(8, 128, CPB)
        return np.ascontiguousarray(wt.transpose(1, 0, 2))

    in_maps = []
    for c in range(NCORES):
        s = slice(c * CPB, (c + 1) * CPB)
        wp = weffs["Wproj"][s, :].astype(BF16_NP)   # [CPB, D]
        in_maps.append({
            "xqT": xqT, "xkT": xkT, "xvT": xvT,
            "weffq": tile_w(weffs["Wq"][:, s]),
            "weffk": tile_w(weffs["Wk"][:, s]),
            "weffv": tile_w(weffs["Wv"][:, s]),
            "weffp": np.ascontiguousarray(wp.reshape(CPB, 2, 512)),
        })
    return in_maps


def run(inputs, trace=False, trace_cores=None):
    nc = _get_nc()
    in_maps = _prep_in_maps(inputs)
    res = run_bass_kernel_spmd(nc, in_maps, list(range(NCORES)),
                               trace=trace, trace_cores=trace_cores)
    acc = res.results[0]["out"].astype(np.float32).copy()
    for c in range(1, NCORES):
        acc += res.results[c]["out"].astype(np.float32)
    acc += np.asarray(inputs["bproj"], dtype=np.float32)[None, :]
    return acc.reshape(B, N, D), res


def kernel(**inputs):
    out, _ = run(inputs, trace=False)
    return out
